# revision 7
# baseline (speedup 1.0000x reference)
"""Causal self-attention on 8 TRN2 NeuronCores.

Sharding: data-parallel over batch (2) x tensor-parallel over heads (4 heads
per core). Core c handles batch c//4, heads 4*(c%4)..4*(c%4)+3 — i.e. columns
[256*g, 256*(g+1)) of wq/wk/wv and rows [256*g, 256*(g+1)) of wo. Each core
returns a partial output [2048, 1024]; the host sums the 4 partials of each
batch and adds the (bv @ wo + bo) correction (exact because softmax rows sum
to 1).

Per-core kernel (Tile framework, fully unrolled, fp32 storage / fp32r matmul):
  1. x [2048,1024] -> PE-transpose -> xT chunks; project to qT/kT/vT [256,2048]
     (j on partitions). q scaled by 1/8 and biased with bq; k biased with bk.
  2. v_aug: PE-transpose vT back to natural [l, j] layout, interleaved with a
     ones column per head ([128, 65] per (l-chunk, head)) so the AV matmul
     also produces the softmax denominator in row 64.
  3. Attention per (head, 512-wide i-block), scores kept TRANSPOSED
     ([l-chunk=128, i=512]) so softmax reduction lands on the matmul and the
     AV/out-proj matmuls need no further transposes. Causal: lower chunks
     skipped entirely; the 4 diagonal chunks get exp() zeroed via
     gpsimd.affine_select. Normalization: reciprocal of psum row 64,
     partition_broadcast to 64 rows, multiply.
  4. y = attn_outT.T @ wo accumulated over the 2 local j-chunks, per 128-token
     tile, DMA'd out.
"""

import os
import sys

import numpy as np

if "/opt/trn_rl_repo" not in sys.path:
    sys.path.insert(0, "/opt/trn_rl_repo")

import concourse.bass as bass
import concourse.mybir as mybir
import concourse.tile as tile
from concourse import bacc
from concourse.bass_utils import run_bass_kernel_spmd
from concourse.masks import make_identity

# Problem shapes (hardcoded per contract)
B, S, D = 2, 2048, 1024
H, DH = 16, 64
NCORES = 8
GROUPS = 4                  # tensor-parallel groups per batch
HL = H // GROUPS            # 4 local heads
JC = HL * DH                # 256 local head columns
T = S                       # tokens per core (one batch element)

P = 128                     # partitions
TS = 512                    # token block (projection granularity)
NTB = T // TS               # 4 token blocks
NDC = D // P                # 8 contraction chunks
IB = 512                    # attention i-block (query positions)
LCH = P                     # attention l-chunk (key positions)
VA = DH + 1                 # v_aug columns per head (ones column appended)

FP = mybir.dt.float32
FPR = mybir.dt.float32r

NEG_SLOPE = None

_CACHE = {}


def build_nc():
    nc = bacc.Bacc("TRN2", target_bir_lowering=False, debug=False)

    x = nc.dram_tensor("x", [T, D], FP, kind="ExternalInput")
    wq = nc.dram_tensor("wq", [D, JC], FPR, kind="ExternalInput")
    wk = nc.dram_tensor("wk", [D, JC], FPR, kind="ExternalInput")
    wv = nc.dram_tensor("wv", [D, JC], FPR, kind="ExternalInput")
    wo = nc.dram_tensor("wo", [JC, D], FPR, kind="ExternalInput")
    bq = nc.dram_tensor("bq", [JC, 1], FP, kind="ExternalInput")
    bk = nc.dram_tensor("bk", [JC, 1], FP, kind="ExternalInput")
    y = nc.dram_tensor("y", [T, D], FP, kind="ExternalOutput")

    with tile.TileContext(nc) as tc:
        import contextlib

        with contextlib.ExitStack() as ctx:
            singles = ctx.enter_context(tc.tile_pool(name="singles", bufs=1))
            xin_pool = ctx.enter_context(tc.tile_pool(name="xin", bufs=6))
            xt_pool = ctx.enter_context(tc.tile_pool(name="xt", bufs=1))
            exp_pool = ctx.enter_context(tc.tile_pool(name="exp", bufs=4))
            nrm_pool = ctx.enter_context(tc.tile_pool(name="nrm", bufs=3))
            ysb_pool = ctx.enter_context(tc.tile_pool(name="ysb", bufs=3))
            ps = ctx.enter_context(tc.tile_pool(name="ps", bufs=2, space="PSUM"))

            # ---- constants / weights ----
            ident = singles.tile([P, P], FP)
            make_identity(nc, ident)

            wq_sb = singles.tile([P, NDC * JC], FPR, tag="wq")   # chunk c at [JC*c, JC*(c+1))
            wk_sb = singles.tile([P, NDC * JC], FPR, tag="wk")
            wv_sb = singles.tile([P, NDC * JC], FPR, tag="wv")
            for c in range(NDC):
                nc.sync.dma_start(out=wq_sb[:, JC * c:JC * (c + 1)], in_=wq[P * c:P * (c + 1), :])
                nc.sync.dma_start(out=wk_sb[:, JC * c:JC * (c + 1)], in_=wk[P * c:P * (c + 1), :])
                nc.sync.dma_start(out=wv_sb[:, JC * c:JC * (c + 1)], in_=wv[P * c:P * (c + 1), :])
            wo_sb = [singles.tile([P, D], FPR, tag=f"wo{j}", name=f"wo_sb{j}") for j in range(2)]
            for j in range(2):
                nc.sync.dma_start(out=wo_sb[j], in_=wo[P * j:P * (j + 1), :])
            bq_sb = [singles.tile([P, 1], FP, tag=f"bq{j}", name=f"bq_sb{j}") for j in range(2)]
            bk_sb = [singles.tile([P, 1], FP, tag=f"bk{j}", name=f"bk_sb{j}") for j in range(2)]
            for j in range(2):
                nc.sync.dma_start(out=bq_sb[j], in_=bq[P * j:P * (j + 1), :])
                nc.sync.dma_start(out=bk_sb[j], in_=bk[P * j:P * (j + 1), :])

            # persistent activations
            qt_sb = [singles.tile([P, T], FPR, tag=f"qt{j}", name=f"qt_sb{j}") for j in range(2)]
            kt_sb = [singles.tile([P, T], FPR, tag=f"kt{j}", name=f"kt_sb{j}") for j in range(2)]
            vt_sb = [singles.tile([P, T], FP, tag=f"vt{j}", name=f"vt_sb{j}") for j in range(2)]
            ao_sb = [singles.tile([P, T], FPR, tag=f"ao{j}", name=f"ao_sb{j}") for j in range(2)]
            # v_aug: chunk lc at [VA*HL*lc , ...), head h at offset VA*h, ones at +DH
            n_lch = T // LCH
            vaug = singles.tile([P, n_lch * HL * VA], FPR, tag="vaug")
            # Write 1.0 into the per-head ones columns (strided view, col DH of
            # each [*, VA] group). Memset rejects f32r and iota writes raw int
            # bits on HW, so compute 1.0 = in*0 + 1 on DVE from loaded data.
            ones_view = vaug.rearrange("p (c v) -> p c v", v=VA)[:, :, DH]
            nc.vector.tensor_scalar(
                out=ones_view, in0=wq_sb[:, 0:n_lch * HL],
                scalar1=0.0, scalar2=1.0,
                op0=mybir.AluOpType.mult, op1=mybir.AluOpType.add,
            )

            for tb in range(NTB):
                # ---- load x block, transpose to xT ----
                xin = []
                for tsub in range(TS // P):
                    xt_in = xin_pool.tile([P, D], FP, tag="xin")
                    nc.sync.dma_start(
                        out=xt_in, in_=x[TS * tb + P * tsub: TS * tb + P * (tsub + 1), :]
                    )
                    xin.append(xt_in)
                xt = xt_pool.tile([P, NDC * TS], FPR, tag="xt")  # chunk c at [TS*c, ...)
                for c in range(NDC):
                    tr = ps.tile([P, TS], FP, tag="tr")
                    for tsub in range(TS // P):
                        nc.tensor.transpose(
                            tr[:, P * tsub:P * (tsub + 1)],
                            xin[tsub][:, P * c:P * (c + 1)],
                            ident,
                        )
                    nc.scalar.activation(
                        out=xt[:, TS * c:TS * (c + 1)], in_=tr,
                        func=mybir.ActivationFunctionType.Copy,
                    )

                # ---- projections qT/kT/vT for this token block ----
                qt_ps = [ps.tile([P, TS], FP, tag="a", name=f"qt_ps{_j}") for _j in range(2)]
                kt_ps = [ps.tile([P, TS], FP, tag="b", name=f"kt_ps{_j}") for _j in range(2)]
                vt_ps = [ps.tile([P, TS], FP, tag="c", name=f"vt_ps{_j}") for _j in range(2)]
                for c in range(NDC):
                    rhs = (xt[:, TS * c:TS * (c + 1)])
                    st = dict(start=(c == 0), stop=(c == NDC - 1))
                    for j in range(2):
                        nc.tensor.matmul(
                            qt_ps[j], (wq_sb[:, JC * c + P * j:JC * c + P * (j + 1)]), rhs, **st
                        )
                        nc.tensor.matmul(
                            kt_ps[j], (wk_sb[:, JC * c + P * j:JC * c + P * (j + 1)]), rhs, **st
                        )
                        nc.tensor.matmul(
                            vt_ps[j], (wv_sb[:, JC * c + P * j:JC * c + P * (j + 1)]), rhs, **st
                        )
                for j in range(2):
                    # qT = psum/8 + bq ; kT = psum + bk (scale folds 1/sqrt(dh))
                    nc.vector.tensor_scalar(
                        out=qt_sb[j][:, TS * tb:TS * (tb + 1)], in0=qt_ps[j],
                        scalar1=0.125, scalar2=bq_sb[j],
                        op0=mybir.AluOpType.mult, op1=mybir.AluOpType.add,
                    )
                    nc.vector.tensor_scalar(
                        out=kt_sb[j][:, TS * tb:TS * (tb + 1)], in0=kt_ps[j],
                        scalar1=bk_sb[j], scalar2=None, op0=mybir.AluOpType.add,
                    )
                    nc.scalar.activation(
                        out=vt_sb[j][:, TS * tb:TS * (tb + 1)], in_=vt_ps[j],
                        func=mybir.ActivationFunctionType.Copy,
                    )

                # ---- v_aug for this block's l-chunks ----
                for lc in range(4 * tb, 4 * (tb + 1)):
                    tr2 = ps.tile([P, 2 * P], FP, tag="tr")
                    for j in range(2):
                        nc.tensor.transpose(
                            tr2[:, P * j:P * (j + 1)],
                            vt_sb[j][:, LCH * lc:LCH * (lc + 1)],
                            ident,
                        )
                    for h in range(HL):
                        nc.vector.tensor_copy(
                            out=vaug[:, VA * HL * lc + VA * h: VA * HL * lc + VA * h + DH],
                            in_=tr2[:, DH * h:DH * (h + 1)],
                        )

                # ---- attention for i-block == tb ----
                i = tb
                nch = 4 * (i + 1)   # causal chunks
                for h in range(HL):
                    j, ro = divmod(h, 2)
                    ro *= DH
                    av = ps.tile([P, IB], FP, tag="b")
                    for c in range(nch):
                        sc = ps.tile([P, IB], FP, tag="a")
                        nc.tensor.matmul(
                            sc,
                            (kt_sb[j][ro:ro + DH, LCH * c:LCH * (c + 1)]),
                            (qt_sb[j][ro:ro + DH, IB * i:IB * (i + 1)]),
                            start=True, stop=True,
                        )
                        ex = exp_pool.tile([P, IB], FPR, tag="ex")
                        nc.scalar.activation(
                            out=ex, in_=sc, func=mybir.ActivationFunctionType.Exp
                        )
                        if c >= 4 * i:
                            # zero exp() where l > i  (keep  f - p - 128*v >= 0)
                            v = c - 4 * i
                            nc.gpsimd.affine_select(
                                out=ex, in_=ex,
                                compare_op=mybir.AluOpType.is_ge,
                                fill=0.0, base=-P * v,
                                channel_multiplier=-1, pattern=[[1, IB]],
                            )
                        nc.tensor.matmul(
                            av[0:VA, :],
                            (vaug[:, VA * HL * c + VA * h: VA * HL * c + VA * (h + 1)]),
                            (ex),
                            start=(c == 0), stop=(c == nch - 1),
                        )
                    recip = nrm_pool.tile([1, IB], FP, tag="rc")
                    nc.vector.reciprocal(out=recip, in_=av[DH:DH + 1, :])
                    bc = nrm_pool.tile([DH, IB], FP, tag="bc")
                    nc.gpsimd.partition_broadcast(out_ap=bc, in_ap=recip)
                    nc.vector.tensor_mul(
                        out=ao_sb[j][ro:ro + DH, IB * i:IB * (i + 1)],
                        in0=av[0:DH, :], in1=bc,
                    )

                # ---- output projection for this i-block's token tiles ----
                for tt in range(4 * i, 4 * (i + 1)):
                    ysb = ysb_pool.tile([P, D], FP, tag="ysb")
                    for db in range(2):
                        yps = ps.tile([P, IB], FP, tag="c")
                        for j in range(2):
                            nc.tensor.matmul(
                                yps,
                                (ao_sb[j][:, P * tt:P * (tt + 1)]),
                                (wo_sb[j][:, IB * db:IB * (db + 1)]),
                                start=(j == 0), stop=(j == 1),
                            )
                        nc.scalar.activation(
                            out=ysb[:, IB * db:IB * (db + 1)], in_=yps,
                            func=mybir.ActivationFunctionType.Copy,
                        )
                    nc.sync.dma_start(out=y[P * tt:P * (tt + 1), :], in_=ysb)

    nc.compile()
    return nc


def get_nc():
    if "nc" not in _CACHE:
        _CACHE["nc"] = build_nc()
    return _CACHE["nc"]


def kernel(x, wq, bq, wk, bk, wv, bv, wo, bo):
    x = np.ascontiguousarray(np.asarray(x, dtype=np.float32))
    wq = np.asarray(wq, dtype=np.float32)
    wk = np.asarray(wk, dtype=np.float32)
    wv = np.asarray(wv, dtype=np.float32)
    wo = np.asarray(wo, dtype=np.float32)
    bq = np.asarray(bq, dtype=np.float32)
    bk = np.asarray(bk, dtype=np.float32)
    bv = np.asarray(bv, dtype=np.float32)
    bo = np.asarray(bo, dtype=np.float32)

    nc = get_nc()
    in_maps = []
    for core in range(NCORES):
        b, g = divmod(core, GROUPS)
        cs = slice(JC * g, JC * (g + 1))
        in_maps.append({
            "x": np.ascontiguousarray(x[b]),
            "wq": np.ascontiguousarray(wq[:, cs]),
            "wk": np.ascontiguousarray(wk[:, cs]),
            "wv": np.ascontiguousarray(wv[:, cs]),
            "wo": np.ascontiguousarray(wo[cs, :]),
            "bq": np.ascontiguousarray(bq[cs].reshape(JC, 1)),
            "bk": np.ascontiguousarray(bk[cs].reshape(JC, 1)),
        })
    res = run_bass_kernel_spmd(nc, in_maps, list(range(NCORES)))
    _CACHE["last_results"] = res

    out = np.zeros((B, S, D), np.float32)
    for core in range(NCORES):
        out[core // GROUPS] += res.results[core]["y"]
    # bv and bo never pass through softmax nonlinearity: rows of attn sum to 1,
    # so (v + bv) contributes exactly bv @ wo to every output row.
    out += (bv @ wo + bo)[None, None, :]
    return out
